# revision 1
# baseline (speedup 1.0000x reference)
"""Irrep GroupNorm kernel for Trainium2, 8-core SPMD.

Reference op: x[4, 296, 32, 32, 32] splits channel-wise into 4 irrep fields
RS = [(64,1), (32,3), (16,5), (8,7)] (mult m, irrep dim d). Per (sample, field):
  - d==1 field: subtract per-sample mean over all m*S elements
  - norm = sum(field^2) / (m*S); scale = (norm+eps)^-1/2 * weight[per-m]
  - d==1 field: add per-m bias
All reductions are per (sample, field), so we shard (sample x field-pair):
  core 2b+0 ("A"): sample b, fields {0, 2} -> 144 channels (+8 pad) = 19 tiles
  core 2b+1 ("B"): sample b, fields {1, 3} -> 152 channels       = 19 tiles
Each tile is 8 channels = [128 partitions, 2048] f32 (1 MiB), fully SBUF-resident
(19 MiB/core). Zero cross-core communication. The SPMD program is identical on
all cores; per-core differences (field boundaries, counts, mean on/off) are
carried in tiny data inputs (masks / expanded weights / constants).

Program phases (uniform across cores):
  load tiles 0..18; per-tile stats (DVE sum, ACT square+accum) as tiles land
  phi1: after tiles 0..11 -> masked-matmul combine slot0 sumsq + mean sum,
        rsqrt, build per-partition affine, apply+store tiles 0..7
        (tiles 0..7 are always entirely in slot0 on both core types)
  phi2: after tiles 8..18 -> combine slot1 sumsq over tiles 8..18, rsqrt,
        per-tile scale vector via mask rows, apply+store tiles 8..18
"""
import numpy as np
from contextlib import ExitStack

import concourse.bass as bass
import concourse.tile as tile
import concourse.mybir as mybir
from concourse.bass_utils import run_bass_kernel_spmd

F32 = mybir.dt.float32
ALU = mybir.AluOpType
ACTF = mybir.ActivationFunctionType
AXX = mybir.AxisListType.X

S = 32 * 32 * 32          # spatial per channel
T = 19                    # tiles per core
P = 128                   # partitions
FREE = 2048               # 8ch * S / 128
CPT = 8                   # channels per tile
EPS = 1e-5
N_CORES = 8
PHI1 = 12                 # tiles 0..11 feed the phi1 combine
APL1 = 8                  # tiles 0..7 applied in phi1


def _build_program() -> bass.Bass:
    nc = bass.Bass("TRN2", target_bir_lowering=False, debug=False)
    xin = nc.dram_tensor("xin", [T, P, FREE], F32, kind="ExternalInput").ap()
    wexp = nc.dram_tensor("wexp", [P, T], F32, kind="ExternalInput").ap()
    bexp = nc.dram_tensor("bexp", [P, APL1], F32, kind="ExternalInput").ap()
    masks = nc.dram_tensor("masks", [P, 3 * T], F32, kind="ExternalInput").ap()
    maskrow = nc.dram_tensor("maskrow", [1, 2 * (T - APL1)], F32, kind="ExternalInput").ap()
    consts = nc.dram_tensor("consts", [1, 3], F32, kind="ExternalInput").ap()
    yout = nc.dram_tensor("yout", [T, P, FREE], F32, kind="ExternalOutput").ap()

    NP2 = T - APL1  # phi2 tile count (11)

    with ExitStack() as octx, tile.TileContext(nc) as tc:
        ctx = octx.enter_context(ExitStack())
        xp = ctx.enter_context(tc.tile_pool(name="xp", bufs=T))
        sp = ctx.enter_context(tc.tile_pool(name="sp", bufs=T))
        qp = ctx.enter_context(tc.tile_pool(name="qp", bufs=2))
        cp = ctx.enter_context(tc.tile_pool(name="cp", bufs=1))
        pp = ctx.enter_context(tc.tile_pool(name="pp", bufs=1, space="PSUM"))

        wexp_t = cp.tile([P, T], F32, tag="wexp")
        nc.sync.dma_start(wexp_t[:], wexp[:])
        bexp_t = cp.tile([P, APL1], F32, tag="bexp")
        nc.sync.dma_start(bexp_t[:], bexp[:])
        masks_t = cp.tile([P, 3 * T], F32, tag="masks")
        nc.sync.dma_start(masks_t[:], masks[:])
        mrow_t = cp.tile([1, 2 * NP2], F32, tag="mrow")
        nc.sync.dma_start(mrow_t[:], maskrow[:])
        consts_t = cp.tile([1, 3], F32, tag="consts")
        nc.sync.dma_start(consts_t[:], consts[:])
        ones1 = cp.tile([1, P], F32, tag="ones1")
        nc.vector.memset(ones1[:], 1.0)

        xts, sts = [], []
        for t in range(T):
            xt = xp.tile([P, FREE], F32, tag="xt")
            nc.sync.dma_start(xt[:], xin[t])
            st = sp.tile([P, 2], F32, tag="st")
            nc.vector.reduce_sum(st[:, 0:1], xt[:], axis=AXX)
            sqt = qp.tile([P, FREE], F32, tag="sq")
            nc.scalar.activation(sqt[:], xt[:], ACTF.Square, accum_out=st[:, 1:2])
            xts.append(xt)
            sts.append(st)

        # ---- phi1 combine: q0 = masked sum of sumsq, s0 = masked sum of sums
        pq0 = pp.tile([1, 1], F32, tag="pq0")
        ps0 = pp.tile([1, 1], F32, tag="ps0")
        for t in range(PHI1):
            nc.tensor.matmul(pq0[:], lhsT=masks_t[:, 3 * t:3 * t + 1],
                             rhs=sts[t][:, 1:2], start=(t == 0),
                             stop=(t == PHI1 - 1), skip_group_check=True)
            nc.tensor.matmul(ps0[:], lhsT=masks_t[:, 3 * t + 2:3 * t + 3],
                             rhs=sts[t][:, 0:1], start=(t == 0),
                             stop=(t == PHI1 - 1), skip_group_check=True)

        # scalar chain on partition 0: r0 = rsqrt(q0*inv0 - mu^2 + EPS), mu = s0*invmean
        t0 = cp.tile([1, 1], F32, tag="t0")
        nc.vector.tensor_mul(t0[:], pq0[:], consts_t[:, 0:1])
        mu = cp.tile([1, 1], F32, tag="mu")
        nc.vector.tensor_mul(mu[:], ps0[:], consts_t[:, 2:3])
        mu2 = cp.tile([1, 1], F32, tag="mu2")
        nc.vector.tensor_mul(mu2[:], mu[:], mu[:])
        v0 = cp.tile([1, 1], F32, tag="v0")
        nc.vector.tensor_sub(v0[:], t0[:], mu2[:])
        v0e = cp.tile([1, 1], F32, tag="v0e")
        nc.vector.tensor_scalar_add(v0e[:], v0[:], EPS)
        rec0 = cp.tile([1, 1], F32, tag="rec0")
        nc.vector.reciprocal(rec0[:], v0e[:])
        rm = cp.tile([1, 2], F32, tag="rm")  # [r0, -mu]
        nc.scalar.sqrt(rm[:, 0:1], rec0[:])
        nc.vector.tensor_scalar_mul(rm[:, 1:2], mu[:], -1.0)

        prm = pp.tile([P, 2], F32, tag="prm")  # broadcast [r0, -mu] to all partitions
        nc.tensor.matmul(prm[:], lhsT=ones1[:], rhs=rm[:], start=True, stop=True,
                         skip_group_check=True)
        a1 = cp.tile([P, APL1], F32, tag="a1")
        nc.vector.tensor_scalar_mul(a1[:], wexp_t[:, 0:APL1], prm[:, 0:1])
        b1 = cp.tile([P, APL1], F32, tag="b1")
        nc.vector.scalar_tensor_tensor(b1[:], in0=a1[:], scalar=prm[:, 1:2],
                                       in1=bexp_t[:], op0=ALU.mult, op1=ALU.add)
        for t in range(APL1):
            nc.vector.tensor_scalar(xts[t][:], xts[t][:], a1[:, t:t + 1],
                                    b1[:, t:t + 1], ALU.mult, ALU.add)
            nc.sync.dma_start(yout[t], xts[t][:])

        # ---- phi2 combine: q1 over tiles 8..18
        pq1 = pp.tile([1, 1], F32, tag="pq1")
        for t in range(APL1, T):
            nc.tensor.matmul(pq1[:], lhsT=masks_t[:, 3 * t + 1:3 * t + 2],
                             rhs=sts[t][:, 1:2], start=(t == APL1),
                             stop=(t == T - 1), skip_group_check=True)
        t1 = cp.tile([1, 1], F32, tag="t1")
        nc.vector.tensor_mul(t1[:], pq1[:], consts_t[:, 1:2])
        v1e = cp.tile([1, 1], F32, tag="v1e")
        nc.vector.tensor_scalar_add(v1e[:], t1[:], EPS)
        rec1 = cp.tile([1, 1], F32, tag="rec1")
        nc.vector.reciprocal(rec1[:], v1e[:])
        r1 = cp.tile([1, 1], F32, tag="r1")
        nc.scalar.sqrt(r1[:], rec1[:])

        # per-tile rsqrt row: rv = m0row*r0 + m1row*r1  (tiles 8..18)
        rva = cp.tile([1, NP2], F32, tag="rva")
        nc.vector.tensor_scalar_mul(rva[:], mrow_t[:, 0:NP2], rm[:, 0:1])
        rv = cp.tile([1, NP2], F32, tag="rv")
        nc.vector.scalar_tensor_tensor(rv[:], in0=mrow_t[:, NP2:2 * NP2], scalar=r1[:],
                                       in1=rva[:], op0=ALU.mult, op1=ALU.add)
        prv = pp.tile([P, NP2], F32, tag="prv")
        nc.tensor.matmul(prv[:], lhsT=ones1[:], rhs=rv[:], start=True, stop=True,
                         skip_group_check=True)
        a2 = cp.tile([P, NP2], F32, tag="a2")
        nc.vector.tensor_mul(a2[:], wexp_t[:, APL1:T], prv[:])
        for i, t in enumerate(range(APL1, T)):
            if i % 2 == 0:
                nc.vector.tensor_scalar_mul(xts[t][:], xts[t][:], a2[:, i:i + 1])
            else:
                nc.scalar.activation(xts[t][:], xts[t][:], ACTF.Copy, bias=0.0,
                                     scale=a2[:, i:i + 1])
            nc.sync.dma_start(yout[t], xts[t][:])
        ctx.close()
    return nc


def _per_channel_params(weight: np.ndarray, bias: np.ndarray):
    w = np.concatenate([
        weight[0:64],
        np.repeat(weight[64:96], 3),
        np.repeat(weight[96:112], 5),
        np.repeat(weight[112:120], 7),
    ]).astype(np.float32)
    return w, bias.astype(np.float32)


def _core_meta(g: int):
    """g=0: A-core (fields 0,2), g=1: B-core (fields 1,3). Returns
    (channel ranges, n_real_tiles, m0, m1, mm, inv0, inv1, invmean)."""
    if g == 0:
        rngs = [(0, 64), (160, 240)]
        nreal = 18
        m0 = (np.arange(T) < 8).astype(np.float32)
        m1 = ((np.arange(T) >= 8) & (np.arange(T) < 18)).astype(np.float32)
        mm = m0.copy()
        inv0, inv1, invmean = 1.0 / (64 * S), 1.0 / (16 * S), 1.0 / (64 * S)
    else:
        rngs = [(64, 160), (240, 296)]
        nreal = 19
        m0 = (np.arange(T) < 12).astype(np.float32)
        m1 = (np.arange(T) >= 12).astype(np.float32)
        mm = np.zeros(T, np.float32)
        inv0, inv1, invmean = 1.0 / (32 * S), 1.0 / (8 * S), 0.0
    return rngs, nreal, m0, m1, mm, inv0, inv1, invmean


def _shard(x: np.ndarray, weight: np.ndarray, bias: np.ndarray):
    wch, bch = _per_channel_params(weight, bias)
    xf = x.reshape(4, 296, S)
    in_maps = []
    for core in range(N_CORES):
        b, g = core // 2, core % 2
        rngs, nreal, m0, m1, mm, inv0, inv1, invmean = _core_meta(g)
        xc = np.concatenate([xf[b, lo:hi] for lo, hi in rngs], axis=0)
        xin = np.zeros((T, P, FREE), np.float32)
        xin[:nreal] = xc.reshape(nreal, P, FREE)

        wcore = np.zeros(T * CPT, np.float32)
        wcore[:nreal * CPT] = np.concatenate([wch[lo:hi] for lo, hi in rngs])
        wexp = np.repeat(wcore.reshape(T, CPT), P // CPT, axis=1).T.copy()

        if g == 0:
            bexp = np.repeat(bch.reshape(APL1, CPT), P // CPT, axis=1).T.copy()
        else:
            bexp = np.zeros((P, APL1), np.float32)

        masks = np.zeros((P, 3 * T), np.float32)
        masks[:, 0::3] = m0
        masks[:, 1::3] = m1
        masks[:, 2::3] = mm
        maskrow = np.concatenate([m0[APL1:], m1[APL1:]]).reshape(1, -1)
        consts = np.array([[inv0, inv1, invmean]], np.float32)

        in_maps.append({
            "xin": np.ascontiguousarray(xin),
            "wexp": np.ascontiguousarray(wexp),
            "bexp": np.ascontiguousarray(bexp),
            "masks": np.ascontiguousarray(masks),
            "maskrow": np.ascontiguousarray(maskrow),
            "consts": consts,
        })
    return in_maps


def _unshard(results) -> np.ndarray:
    y = np.empty((4, 296, S), np.float32)
    for core in range(N_CORES):
        b, g = core // 2, core % 2
        rngs, nreal, *_ = _core_meta(g)
        r = results[core]["yout"].reshape(T * CPT, S)
        ofs = 0
        for lo, hi in rngs:
            n = hi - lo
            y[b, lo:hi] = r[ofs:ofs + n]
            ofs += n
    return y.reshape(4, 296, 32, 32, 32)


def run(inputs: dict, **spmd_kwargs):
    x = np.asarray(inputs["x"], dtype=np.float32)
    weight = np.asarray(inputs["weight"], dtype=np.float32)
    bias = np.asarray(inputs["bias"], dtype=np.float32)
    nc = _build_program()
    in_maps = _shard(x, weight, bias)
    res = run_bass_kernel_spmd(nc, in_maps, list(range(N_CORES)), **spmd_kwargs)
    return _unshard(res.results), res


def kernel(**inputs) -> np.ndarray:
    y, _ = run(inputs)
    return y


# revision 3
# speedup vs baseline: 1.5180x; 1.5180x over previous
"""Irrep GroupNorm kernel for Trainium2, 8-core SPMD.

Reference op: x[4, 296, 32, 32, 32] splits channel-wise into 4 irrep fields
RS = [(64,1), (32,3), (16,5), (8,7)] (mult m, irrep dim d). Per (sample, field):
  - d==1 field: subtract per-sample mean over all m*S elements
  - norm = sum(field^2) / (m*S); scale = (norm+eps)^-1/2 * weight[per-m]
  - d==1 field: add per-m bias
All reductions are per (sample, field), so we shard (sample x field-pair):
  core 2b+0 ("A"): sample b, fields {0, 2} -> 144 channels (+8 pad) = 19 tiles
  core 2b+1 ("B"): sample b, fields {1, 3} -> 152 channels       = 19 tiles
Each tile is 8 channels = [128 partitions, 2048] f32 (1 MiB), fully SBUF-resident
(19 MiB/core). Zero cross-core communication. The SPMD program is identical on
all cores; per-core differences (field boundaries, counts, mean on/off) are
carried in one small per-core "params" data tensor (masks / expanded weights /
inverse counts).

Program phases (uniform across cores):
  load tiles 0..18 (alternating SP/Pool DMA queues so queue-prep delays hide
  under transfers); per-tile stats (DVE sum, ACT square+accum) as tiles land
  phi1: after tiles 0..11 -> masked-matmul combine slot0 sumsq + mean sum,
        rsqrt, build per-partition affine, apply+store tiles 0..7
        (tiles 0..7 are always entirely in slot0 on both core types)
  phi2: after tiles 8..18 -> combine slot1 sumsq over tiles 8..18, rsqrt,
        per-tile scale vector via mask rows, apply+store tiles 8..18
"""
import numpy as np
from contextlib import ExitStack

import concourse.bacc as bacc
import concourse.tile as tile
import concourse.mybir as mybir
from concourse.bass_utils import run_bass_kernel_spmd

F32 = mybir.dt.float32
ALU = mybir.AluOpType
ACTF = mybir.ActivationFunctionType
AXX = mybir.AxisListType.X

S = 32 * 32 * 32          # spatial per channel
T = 19                    # tiles per core
P = 128                   # partitions
FREE = 2048               # 8ch * S / 128
CPT = 8                   # channels per tile
EPS = 1e-5
N_CORES = 8
PHI1 = 12                 # tiles 0..11 feed the phi1 combine
APL1 = 8                  # tiles 0..7 applied in phi1
NP2 = T - APL1            # phi2 tile count (11)

# params tensor column layout
PC_W = 0                  # [P, 0:19]    wexp
PC_B = T                  # [P, 19:27]   bexp
PC_M = PC_B + APL1        # [P, 27:84]   masks, col 3t+{0,1,2} = m0,m1,mm
PC_ONES = PC_M + 3 * T    # [p0, 84:212] ones row (for K=1 broadcast matmuls)
PC_MR = PC_ONES + P       # [p0, 212:234] maskrow: m0[8:19], m1[8:19]
PC_C = PC_MR + 2 * NP2    # [p0, 234:237] consts: inv0, inv1, invmean
PCOLS = 240


def _build_program():
    nc = bacc.Bacc("TRN2", target_bir_lowering=False, debug=False)
    xin = nc.dram_tensor("xin", [T, P, FREE], F32, kind="ExternalInput").ap()
    params = nc.dram_tensor("params", [P, PCOLS], F32, kind="ExternalInput").ap()
    yout = nc.dram_tensor("yout", [T, P, FREE], F32, kind="ExternalOutput").ap()

    def io_eng(t):
        return nc.sync if t % 2 == 0 else nc.gpsimd

    with ExitStack() as octx, tile.TileContext(nc) as tc:
        ctx = octx.enter_context(ExitStack())
        xp = ctx.enter_context(tc.tile_pool(name="xp", bufs=T))
        sump = ctx.enter_context(tc.tile_pool(name="sump", bufs=T))
        sqp = ctx.enter_context(tc.tile_pool(name="sqp", bufs=T))
        qp = ctx.enter_context(tc.tile_pool(name="qp", bufs=2))
        cp = ctx.enter_context(tc.tile_pool(name="cp", bufs=1))
        pp = ctx.enter_context(tc.tile_pool(name="pp", bufs=1, space="PSUM"))

        pt = cp.tile([P, PCOLS], F32, tag="pt")
        nc.scalar.dma_start(pt[:], params[:])
        wexp = pt[:, PC_W:PC_W + T]
        bexp = pt[:, PC_B:PC_B + APL1]
        masks = pt[:, PC_M:PC_M + 3 * T]
        ones1 = pt[0:1, PC_ONES:PC_ONES + P]
        mrow = pt[0:1, PC_MR:PC_MR + 2 * NP2]
        consts = pt[0:1, PC_C:PC_C + 3]

        xts, sums, sqs = [], [], []
        for t in range(T):
            xt = xp.tile([P, FREE], F32, tag="xt")
            io_eng(t).dma_start(xt[:], xin[t])
            sm = sump.tile([P, 1], F32, tag="sm")
            nc.vector.reduce_sum(sm[:], xt[:], axis=AXX)
            sq = sqp.tile([P, 1], F32, tag="sqst")
            sqt = qp.tile([P, FREE], F32, tag="sq")
            nc.scalar.activation(sqt[:], xt[:], ACTF.Square, accum_out=sq[:])
            xts.append(xt)
            sums.append(sm)
            sqs.append(sq)

        # ---- phi1 combine: q0 = masked sum of sumsq, s0 = masked sum of sums
        pq0 = pp.tile([1, 1], F32, tag="pq0")
        ps0 = pp.tile([1, 1], F32, tag="ps0")
        for t in range(PHI1):
            nc.tensor.matmul(pq0[:], lhsT=masks[:, 3 * t:3 * t + 1],
                             rhs=sqs[t][:], start=(t == 0),
                             stop=(t == PHI1 - 1), skip_group_check=True)
            nc.tensor.matmul(ps0[:], lhsT=masks[:, 3 * t + 2:3 * t + 3],
                             rhs=sums[t][:], start=(t == 0),
                             stop=(t == PHI1 - 1), skip_group_check=True)

        # scalar chain on partition 0: r0 = rsqrt(q0*inv0 - mu^2 + EPS), mu = s0*invmean
        t0 = cp.tile([1, 1], F32, tag="t0")
        nc.vector.tensor_mul(t0[:], pq0[:], consts[:, 0:1])
        mu = cp.tile([1, 1], F32, tag="mu")
        nc.vector.tensor_mul(mu[:], ps0[:], consts[:, 2:3])
        mu2 = cp.tile([1, 1], F32, tag="mu2")
        nc.vector.tensor_mul(mu2[:], mu[:], mu[:])
        v0 = cp.tile([1, 1], F32, tag="v0")
        nc.vector.tensor_sub(v0[:], t0[:], mu2[:])
        v0e = cp.tile([1, 1], F32, tag="v0e")
        nc.vector.tensor_scalar_add(v0e[:], v0[:], EPS)
        rec0 = cp.tile([1, 1], F32, tag="rec0")
        nc.vector.reciprocal(rec0[:], v0e[:])
        rm = cp.tile([1, 2], F32, tag="rm")  # [r0, -mu], single writer: ACT
        nc.scalar.sqrt(rm[:, 0:1], rec0[:])
        nc.scalar.mul(rm[:, 1:2], mu[:], -1.0)

        prm = pp.tile([P, 2], F32, tag="prm")  # broadcast [r0, -mu] to all partitions
        nc.tensor.matmul(prm[:], lhsT=ones1[:], rhs=rm[:], start=True, stop=True,
                         skip_group_check=True)
        a1 = cp.tile([P, APL1], F32, tag="a1")
        nc.vector.tensor_scalar_mul(a1[:], wexp[:, 0:APL1], prm[:, 0:1])
        b1 = cp.tile([P, APL1], F32, tag="b1")
        nc.vector.scalar_tensor_tensor(b1[:], in0=a1[:], scalar=prm[:, 1:2],
                                       in1=bexp[:], op0=ALU.mult, op1=ALU.add)
        for t in range(APL1):
            nc.vector.tensor_scalar(xts[t][:], xts[t][:], a1[:, t:t + 1],
                                    b1[:, t:t + 1], ALU.mult, ALU.add)
            io_eng(t).dma_start(yout[t], xts[t][:])

        # ---- phi2 combine: q1 over tiles 8..18
        pq1 = pp.tile([1, 1], F32, tag="pq1")
        for t in range(APL1, T):
            nc.tensor.matmul(pq1[:], lhsT=masks[:, 3 * t + 1:3 * t + 2],
                             rhs=sqs[t][:], start=(t == APL1),
                             stop=(t == T - 1), skip_group_check=True)
        t1 = cp.tile([1, 1], F32, tag="t1")
        nc.vector.tensor_mul(t1[:], pq1[:], consts[:, 1:2])
        v1e = cp.tile([1, 1], F32, tag="v1e")
        nc.vector.tensor_scalar_add(v1e[:], t1[:], EPS)
        rec1 = cp.tile([1, 1], F32, tag="rec1")
        nc.vector.reciprocal(rec1[:], v1e[:])
        r1 = cp.tile([1, 1], F32, tag="r1")
        nc.scalar.sqrt(r1[:], rec1[:])

        # per-tile rsqrt row: rv = m0row*r0 + m1row*r1  (tiles 8..18)
        rva = cp.tile([1, NP2], F32, tag="rva")
        nc.vector.tensor_scalar_mul(rva[:], mrow[:, 0:NP2], rm[:, 0:1])
        rv = cp.tile([1, NP2], F32, tag="rv")
        nc.vector.scalar_tensor_tensor(rv[:], in0=mrow[:, NP2:2 * NP2], scalar=r1[:],
                                       in1=rva[:], op0=ALU.mult, op1=ALU.add)
        prv = pp.tile([P, NP2], F32, tag="prv")
        nc.tensor.matmul(prv[:], lhsT=ones1[:], rhs=rv[:], start=True, stop=True,
                         skip_group_check=True)
        a2 = cp.tile([P, NP2], F32, tag="a2")
        nc.vector.tensor_mul(a2[:], wexp[:, APL1:T], prv[:])
        for i, t in enumerate(range(APL1, T)):
            if i % 2 == 0:
                nc.vector.tensor_scalar_mul(xts[t][:], xts[t][:], a2[:, i:i + 1])
            else:
                nc.scalar.activation(xts[t][:], xts[t][:], ACTF.Copy, bias=0.0,
                                     scale=a2[:, i:i + 1])
            io_eng(t).dma_start(yout[t], xts[t][:])
        ctx.close()
    return nc


def _per_channel_params(weight: np.ndarray, bias: np.ndarray):
    w = np.concatenate([
        weight[0:64],
        np.repeat(weight[64:96], 3),
        np.repeat(weight[96:112], 5),
        np.repeat(weight[112:120], 7),
    ]).astype(np.float32)
    return w, bias.astype(np.float32)


def _core_meta(g: int):
    """g=0: A-core (fields 0,2), g=1: B-core (fields 1,3). Returns
    (channel ranges, n_real_tiles, m0, m1, mm, inv0, inv1, invmean)."""
    if g == 0:
        rngs = [(0, 64), (160, 240)]
        nreal = 18
        m0 = (np.arange(T) < 8).astype(np.float32)
        m1 = ((np.arange(T) >= 8) & (np.arange(T) < 18)).astype(np.float32)
        mm = m0.copy()
        inv0, inv1, invmean = 1.0 / (64 * S), 1.0 / (16 * S), 1.0 / (64 * S)
    else:
        rngs = [(64, 160), (240, 296)]
        nreal = 19
        m0 = (np.arange(T) < 12).astype(np.float32)
        m1 = (np.arange(T) >= 12).astype(np.float32)
        mm = np.zeros(T, np.float32)
        inv0, inv1, invmean = 1.0 / (32 * S), 1.0 / (8 * S), 0.0
    return rngs, nreal, m0, m1, mm, inv0, inv1, invmean


def _shard(x: np.ndarray, weight: np.ndarray, bias: np.ndarray):
    wch, bch = _per_channel_params(weight, bias)
    xf = x.reshape(4, 296, S)
    in_maps = []
    for core in range(N_CORES):
        b, g = core // 2, core % 2
        rngs, nreal, m0, m1, mm, inv0, inv1, invmean = _core_meta(g)
        xc = np.concatenate([xf[b, lo:hi] for lo, hi in rngs], axis=0)
        xin = np.zeros((T, P, FREE), np.float32)
        xin[:nreal] = xc.reshape(nreal, P, FREE)

        wcore = np.zeros(T * CPT, np.float32)
        wcore[:nreal * CPT] = np.concatenate([wch[lo:hi] for lo, hi in rngs])
        wexp = np.repeat(wcore.reshape(T, CPT), P // CPT, axis=1).T

        pt = np.zeros((P, PCOLS), np.float32)
        pt[:, PC_W:PC_W + T] = wexp
        if g == 0:
            pt[:, PC_B:PC_B + APL1] = np.repeat(bch.reshape(APL1, CPT), P // CPT, axis=1).T
        pt[:, PC_M + 0:PC_M + 3 * T:3] = m0
        pt[:, PC_M + 1:PC_M + 3 * T:3] = m1
        pt[:, PC_M + 2:PC_M + 3 * T:3] = mm
        pt[0, PC_ONES:PC_ONES + P] = 1.0
        pt[0, PC_MR:PC_MR + 2 * NP2] = np.concatenate([m0[APL1:], m1[APL1:]])
        pt[0, PC_C:PC_C + 3] = [inv0, inv1, invmean]

        in_maps.append({
            "xin": np.ascontiguousarray(xin),
            "params": np.ascontiguousarray(pt),
        })
    return in_maps


def _unshard(results) -> np.ndarray:
    y = np.empty((4, 296, S), np.float32)
    for core in range(N_CORES):
        b, g = core // 2, core % 2
        rngs, nreal, *_ = _core_meta(g)
        r = results[core]["yout"].reshape(T * CPT, S)
        ofs = 0
        for lo, hi in rngs:
            n = hi - lo
            y[b, lo:hi] = r[ofs:ofs + n]
            ofs += n
    return y.reshape(4, 296, 32, 32, 32)


def run(inputs: dict, **spmd_kwargs):
    x = np.asarray(inputs["x"], dtype=np.float32)
    weight = np.asarray(inputs["weight"], dtype=np.float32)
    bias = np.asarray(inputs["bias"], dtype=np.float32)
    nc = _build_program()
    in_maps = _shard(x, weight, bias)
    res = run_bass_kernel_spmd(nc, in_maps, list(range(N_CORES)), **spmd_kwargs)
    return _unshard(res.results), res


def kernel(**inputs) -> np.ndarray:
    y, _ = run(inputs)
    return y
